# revision 1
# baseline (speedup 1.0000x reference)
"""Trainium2 Bass kernel for nn_MetaNetLinearizedModel.

Math (B=16, D=12288, F=768, HID=192, T=8):
    X = x.reshape(B, D)
    h1 = X @ W1 + b1                       [B, F]
    g  = gelu_tanh(h1); gp = gelu_tanh'(h1)
    feats = g @ W2 + b2                    [B, F]
    mh = relu(feats @ mW1.T + mb1)         [B, HID]
    coefs = mh @ mW2.T + mb2               [B, T]
    dh1   = sum_t coefs[:,t] * (X @ dW1[t] + db1[t])
    dout  = (gp * dh1) @ W2 + sum_t coefs[:,t] * (g @ dW2[t] + db2[t])
    out   = feats + dout

Sharding over T (8 task vectors -> 8 cores). Core c owns t=c:
    - h1 partial from a D-shard of W1 (D/8 rows), AllReduce -> full h1
    - U = X @ dW1[c]  (the big stream; bf16 halves the HBM traffic)
    - p_dout[c] = (gp * (U + db1[c])) @ W2 + (g @ dW2[c] + db2[c]),
      all scaled by coefs[:,c] at the end (the scale commutes)
    - host: out = feats + sum_c p_dout[c]

Orientation: outputs live in [B(=16, padded 32) partitions, F free] layout.
Big tensors (dW1, W1-shard, W2, dW2, mW1T) are MOVING operands (N=512/256),
small activation [d|f, b] tiles are stationary.
Precision: h1/feats/base path fp32 (dominates the output); delta paths
(U, V, Z2) bf16; meta-net fp32r.
"""
import sys

sys.path.insert(0, "/opt/trn_rl_repo")

import numpy as np
import ml_dtypes
import concourse.bass as bass
import concourse.bacc as bacc
import concourse.tile as tile
import concourse.mybir as mybir
from concourse import bass_utils

F32 = mybir.dt.float32
F32R = mybir.dt.float32r
BF16 = mybir.dt.bfloat16
AF = mybir.ActivationFunctionType
OP = mybir.AluOpType

B = 16
D = 3 * 64 * 64        # 12288
F = 768
HID = 192
T = 8
NCORES = 8
DSH = D // NCORES      # 1536
KD = D // 128          # 96 k-tiles over D
KJ = KD // 2           # 48 double-tiles
KSH = DSH // 128       # 12
FO = F // 128          # 6
KF = F // 128          # 6
GELU_C0 = float(np.sqrt(2.0 / np.pi))
GELU_A = 0.044715

# bc pack column offsets
BC_B1, BC_B2, BC_DB1, BC_DB2 = 0, F, 2 * F, 3 * F
BC_MB1 = 4 * F                 # width HID
BC_MB2 = BC_MB1 + HID          # width T
BC_SEL = BC_MB2 + T            # width T
BC_EYE = BC_SEL + T            # width 32
BC_W = BC_EYE + 32             # 3312

_CACHE = {}


def build():
    nc = bacc.Bacc("TRN2", target_bir_lowering=False, debug=False,
                   enable_asserts=False, num_devices=NCORES)

    XT = nc.dram_tensor("xt", [128, KD * B], BF16, kind="ExternalInput")
    XTS = nc.dram_tensor("xts", [128, KSH * B], F32, kind="ExternalInput")
    W1S = nc.dram_tensor("w1s", [DSH, F], F32, kind="ExternalInput")
    W2 = nc.dram_tensor("w2", [F, F], F32, kind="ExternalInput")
    DW1 = nc.dram_tensor("dw1", [KJ * 128, 2 * F], BF16, kind="ExternalInput")
    DW2 = nc.dram_tensor("dw2", [F, F], F32R, kind="ExternalInput")
    MW1T = nc.dram_tensor("mw1t", [F, HID], F32R, kind="ExternalInput")
    MW2T = nc.dram_tensor("mw2t", [HID, T], F32R, kind="ExternalInput")
    BC = nc.dram_tensor("bc", [32, BC_W], F32, kind="ExternalInput")
    FS = nc.dram_tensor("fs", [128, B], F32, kind="ExternalInput")
    OT = nc.dram_tensor("ot", [B, F], F32, kind="ExternalOutput")
    FT = nc.dram_tensor("ft", [B, F], F32, kind="ExternalOutput")

    with tile.TileContext(nc, num_cores=NCORES) as tc:
        with (
            tc.tile_pool(name="cst", bufs=1) as cst,
            tc.tile_pool(name="wrk", bufs=1) as wrk,
            tc.tile_pool(name="gtmp", bufs=4) as gtmp,
            tc.tile_pool(name="w1sp", bufs=12) as w1sp,
            tc.tile_pool(name="dw1p", bufs=12) as dw1p,
            tc.tile_pool(name="psu", bufs=1, space="PSUM") as psu,
            tc.tile_pool(name="pss", bufs=4, space="PSUM") as pss,
            tc.tile_pool(name="drm", bufs=1, space="DRAM") as drm,
        ):
            # ---- critical h1 path loads first, on the SYNC ring ----
            xts_sb = cst.tile([128, KSH * B], F32)
            nc.sync.dma_start(xts_sb[:], XTS.ap())

            # h1 partial (fp32), weights streamed on sync ring
            h5 = pss.tile([B, 512], F32, name="ps", tag="ps")
            h2 = pss.tile([B, 256], F32, name="ps", tag="ps")
            for k in range(KSH):
                wt = w1sp.tile([128, F], F32, name="w1t", tag="w1t")
                eng = nc.scalar if k % 2 == 0 else nc.sync
                eng.dma_start(wt[:], W1S.ap()[k * 128:(k + 1) * 128, :])
                nc.tensor.matmul(h5[:], xts_sb[:, k * B:(k + 1) * B],
                                 wt[:, 0:512],
                                 start=(k == 0), stop=(k == KSH - 1))
                nc.tensor.matmul(h2[:], xts_sb[:, k * B:(k + 1) * B],
                                 wt[:, 512:F],
                                 start=(k == 0), stop=(k == KSH - 1))
            h1p_sb = wrk.tile([32, F], F32)
            nc.vector.tensor_copy(h1p_sb[0:B, 0:512], h5[:])
            nc.vector.tensor_copy(h1p_sb[0:B, 512:F], h2[:])
            h1p_d = drm.tile([B, F], F32)
            h1ar_d = drm.tile([B, F], F32)
            nc.gpsimd.dma_start(h1p_d[:], h1p_sb[0:B, :])
            nc.gpsimd.collective_compute(
                "AllReduce", OP.add,
                replica_groups=[list(range(NCORES))],
                ins=[h1p_d[:]], outs=[h1ar_d[:]])
            h1r_sb = wrk.tile([32, F], F32)
            nc.gpsimd.dma_start(h1r_sb[0:B, :], h1ar_d[:])

            # ---- remaining loads: xt + dW1 stream on sync; rest on scalar ----
            xt_sb = cst.tile([128, KD * B], BF16)
            nc.sync.dma_start(xt_sb[:], XT.ap())

            bc_sb = cst.tile([32, BC_W], F32)
            nc.scalar.dma_start(bc_sb[:], BC.ap())
            w2_sb = cst.tile([128, KF * F], F32)
            for k in range(KF):
                nc.scalar.dma_start(
                    w2_sb[:, k * F:(k + 1) * F],
                    W2.ap()[k * 128:(k + 1) * 128, :])
            mw1t_sb = cst.tile([128, KF * HID], F32R)
            for k in range(KF):
                nc.scalar.dma_start(
                    mw1t_sb[:, k * HID:(k + 1) * HID],
                    MW1T.ap()[k * 128:(k + 1) * 128, :])
            mw2t_sb = cst.tile([128, 2 * T], F32R)
            nc.scalar.dma_start(mw2t_sb[:, 0:T], MW2T.ap()[0:128, :])
            nc.scalar.dma_start(mw2t_sb[0:HID - 128, T:2 * T],
                                MW2T.ap()[128:HID, :])
            dw2_sb = cst.tile([128, KF * F], F32R)
            for k in range(KF):
                nc.scalar.dma_start(
                    dw2_sb[:, k * F:(k + 1) * F],
                    DW2.ap()[k * 128:(k + 1) * 128, :])
            # fp32r copy of W2 for the (error-tolerant) Z2 path
            w2r_sb = cst.tile([128, KF * F], F32R)
            nc.vector.tensor_copy(w2r_sb[:], w2_sb[:])

            def bcs(col, w):
                return bc_sb[0:B, col:col + w]
            eye = bc_sb[0:32, BC_EYE:BC_EYE + 32]

            # ---- h1 = allreduce(h1p) + b1; gelu chain (rows 0:B) ----
            h1_bt = wrk.tile([32, F], F32)
            nc.vector.tensor_add(h1_bt[0:B, :], h1r_sb[0:B, :],
                                 bcs(BC_B1, F))

            def gt():
                return gtmp.tile([32, F], F32, name="gt", tag="gt")
            h1 = h1_bt[0:B, :]
            s_ = gt(); nc.vector.tensor_mul(s_[0:B, :], h1, h1)
            t1 = gt(); nc.vector.scalar_tensor_tensor(
                t1[0:B, :], s_[0:B, :], GELU_A, h1, OP.mult, OP.mult)
            up = gt(); nc.vector.tensor_add(up[0:B, :], h1, t1[0:B, :])
            t_sb = wrk.tile([32, F], F32)
            nc.scalar.activation(t_sb[0:B, :], up[0:B, :], AF.Tanh,
                                 scale=GELU_C0)
            tt = t_sb[0:B, :]
            t2 = gt(); nc.vector.tensor_mul(t2[0:B, :], tt, tt)
            q_ = gt(); nc.vector.tensor_scalar(q_[0:B, :], t2[0:B, :],
                                               -1.0, 1.0, OP.mult, OP.add)
            r_ = gt(); nc.vector.tensor_scalar(r_[0:B, :], s_[0:B, :],
                                               3.0 * GELU_A, 1.0,
                                               OP.mult, OP.add)
            m_ = gt(); nc.vector.tensor_mul(m_[0:B, :], q_[0:B, :], r_[0:B, :])
            n_ = gt(); nc.vector.tensor_mul(n_[0:B, :], m_[0:B, :], h1)
            th_sb = wrk.tile([32, F], F32)
            nc.vector.tensor_scalar(th_sb[0:B, :], tt, 0.5, 0.5,
                                    OP.mult, OP.add)
            gp_bt = wrk.tile([32, F], F32)
            nc.vector.scalar_tensor_tensor(
                gp_bt[0:B, :], n_[0:B, :], 0.5 * GELU_C0, th_sb[0:B, :],
                OP.mult, OP.add)
            g_bt = wrk.tile([32, F], F32)
            nc.vector.memset(g_bt[:], 0.0)
            nc.vector.tensor_mul(g_bt[0:B, :], th_sb[0:B, :], h1)

            # ---- transpose g -> [f, b] stationary tiles ----
            g_t = wrk.tile([128, FO * B], F32)
            g_tr = wrk.tile([128, FO * B], F32R)
            for fo in range(FO):
                tp = pss.tile([128, 32], F32, name="ps", tag="ps")
                nc.tensor.transpose(tp[:], g_bt[0:32, fo * 128:(fo + 1) * 128],
                                    eye)
                nc.vector.tensor_copy(g_t[:, fo * B:(fo + 1) * B], tp[:, 0:B])
                nc.vector.tensor_copy(g_tr[:, fo * B:(fo + 1) * B], tp[:, 0:B])

            # ---- feats = g @ W2 + b2 (fp32) ----
            f5 = pss.tile([B, 512], F32, name="ps", tag="ps")
            f2 = pss.tile([B, 256], F32, name="ps", tag="ps")
            for k in range(KF):
                nc.tensor.matmul(f5[:], g_t[:, k * B:(k + 1) * B],
                                 w2_sb[:, k * F:k * F + 512],
                                 start=(k == 0), stop=(k == KF - 1))
                nc.tensor.matmul(f2[:], g_t[:, k * B:(k + 1) * B],
                                 w2_sb[:, k * F + 512:(k + 1) * F],
                                 start=(k == 0), stop=(k == KF - 1))
            feats_bt = wrk.tile([32, F], F32)
            nc.vector.memset(feats_bt[:], 0.0)
            nc.vector.tensor_add(feats_bt[0:B, 0:512], f5[:], bcs(BC_B2, 512))
            nc.vector.tensor_add(feats_bt[0:B, 512:F], f2[:],
                                 bc_sb[0:B, BC_B2 + 512:BC_B2 + F])
            nc.scalar.dma_start(FT.ap(), feats_bt[0:B, :])

            feats_tr = wrk.tile([128, FO * B], F32R)
            for fo in range(FO):
                tp = pss.tile([128, 32], F32, name="ps", tag="ps")
                nc.tensor.transpose(tp[:],
                                    feats_bt[0:32, fo * 128:(fo + 1) * 128],
                                    eye)
                nc.vector.tensor_copy(feats_tr[:, fo * B:(fo + 1) * B],
                                      tp[:, 0:B])

            # ---- meta-net (fp32r) ----
            mps = pss.tile([B, HID], F32, name="ps", tag="ps")
            for k in range(KF):
                nc.tensor.matmul(mps[:], feats_tr[:, k * B:(k + 1) * B],
                                 mw1t_sb[:, k * HID:(k + 1) * HID],
                                 start=(k == 0), stop=(k == KF - 1))
            mh_bt = wrk.tile([32, HID], F32)
            nc.vector.memset(mh_bt[:], 0.0)
            mtmp = gtmp.tile([32, F], F32, name="gt", tag="gt")
            nc.vector.tensor_add(mtmp[0:B, 0:HID], mps[:], bcs(BC_MB1, HID))
            nc.vector.tensor_relu(mh_bt[0:B, :], mtmp[0:B, 0:HID])

            mh_tr = wrk.tile([128, 2 * B], F32R)
            tp = pss.tile([128, 32], F32, name="ps", tag="ps")
            nc.tensor.transpose(tp[:], mh_bt[0:32, 0:128], eye)
            nc.vector.tensor_copy(mh_tr[:, 0:B], tp[:, 0:B])
            tp = pss.tile([128, 32], F32, name="ps", tag="ps")
            nc.tensor.transpose(tp[0:HID - 128, :], mh_bt[0:32, 128:HID], eye)
            nc.vector.tensor_copy(mh_tr[0:HID - 128, B:2 * B],
                                  tp[0:HID - 128, 0:B])

            cps = pss.tile([B, T], F32, name="ps", tag="ps")
            nc.tensor.matmul(cps[:], mh_tr[:, 0:B], mw2t_sb[:, 0:T],
                             start=True, stop=False)
            nc.tensor.matmul(cps[:], mh_tr[0:HID - 128, B:2 * B],
                             mw2t_sb[0:HID - 128, T:2 * T],
                             start=False, stop=True)
            coefs_bt = wrk.tile([B, T], F32)
            nc.vector.tensor_add(coefs_bt[:], cps[:], bcs(BC_MB2, T))
            csel = wrk.tile([B, 1], F32)
            cjunk = wrk.tile([B, T], F32)
            nc.vector.tensor_mul(cjunk[:], coefs_bt[:], bcs(BC_SEL, T))
            nc.vector.reduce_sum(csel[:], cjunk[:], axis=mybir.AxisListType.X)

            # ---- V = g @ dW2 + db2 (bf16) ----
            v5 = pss.tile([B, 512], F32, name="ps", tag="ps")
            v2 = pss.tile([B, 256], F32, name="ps", tag="ps")
            for k in range(KF):
                nc.tensor.matmul(v5[:], g_tr[:, k * B:(k + 1) * B],
                                 dw2_sb[:, k * F:k * F + 512],
                                 start=(k == 0), stop=(k == KF - 1))
                nc.tensor.matmul(v2[:], g_tr[:, k * B:(k + 1) * B],
                                 dw2_sb[:, k * F + 512:(k + 1) * F],
                                 start=(k == 0), stop=(k == KF - 1))
            v_bt = wrk.tile([32, F], F32)
            nc.vector.tensor_add(v_bt[0:B, 0:512], v5[:], bcs(BC_DB2, 512))
            nc.vector.tensor_add(v_bt[0:B, 512:F], v2[:],
                                 bc_sb[0:B, BC_DB2 + 512:BC_DB2 + F])

            # ---- U = X @ dW1 (bf16 big stream, sync ring) ----
            # 4 d-tiles run concurrently in PE column-groups 0/32/64/96;
            # partition-group partial sums folded by a selection matmul.
            fs_sb = cst.tile([128, B], F32)
            nc.scalar.dma_start(fs_sb[:], FS.ap())
            u5 = psu.tile([128, 512], F32, name="u5")
            u2 = psu.tile([128, 256], F32, name="u2")

            pair = []
            for j in range(KJ):
                dt_ = dw1p.tile([128, 2 * F], BF16, name="dw1t", tag="dw1t")
                if j >= 10:
                    # hold the bulk stream until the AllReduce result has
                    # landed, so the tiny collective isn't starved by the
                    # bulk SDMA traffic
                    nc.vector.tensor_copy(dt_[0:1, 0:1], h1r_sb[0:1, 0:1])
                nc.sync.dma_start(dt_[:], DW1.ap()[j * 128:(j + 1) * 128, :])
                pair.append(dt_)
                if j % 2 == 0:
                    continue
                i = j // 2
                for cg in range(4):
                    d = 4 * i + cg
                    src_t = pair[cg // 2]
                    half = d % 2
                    nc.tensor.matmul(
                        u5[32 * cg:32 * cg + B, :],
                        xt_sb[:, d * B:(d + 1) * B],
                        src_t[:, half * F:half * F + 512],
                        start=(i == 0), stop=(i == KD // 4 - 1),
                        tile_position=(0, 32 * cg),
                        skip_group_check=True)
                    nc.tensor.matmul(
                        u2[32 * cg:32 * cg + B, :],
                        xt_sb[:, d * B:(d + 1) * B],
                        src_t[:, half * F + 512:(half + 1) * F],
                        start=(i == 0), stop=(i == KD // 4 - 1),
                        tile_position=(0, 32 * cg),
                        skip_group_check=True)
                pair = []
            # fold the 4 partition groups: uf[b, f] = sum_g u[32g+b, f]
            u5_sb = wrk.tile([128, 512], F32)
            u2_sb = wrk.tile([128, 256], F32)
            nc.vector.tensor_copy(u5_sb[:], u5[:])
            nc.vector.tensor_copy(u2_sb[:], u2[:])
            uf5 = pss.tile([B, 512], F32, name="ps", tag="ps")
            uf2 = pss.tile([B, 256], F32, name="ps", tag="ps")
            nc.tensor.matmul(uf5[:], fs_sb[:], u5_sb[:], start=True, stop=True)
            nc.tensor.matmul(uf2[:], fs_sb[:], u2_sb[:], start=True, stop=True)

            # ---- tail ----
            z1_bt = wrk.tile([32, F], F32)
            nc.vector.memset(z1_bt[:], 0.0)
            tz = gtmp.tile([32, F], F32, name="gt", tag="gt")
            nc.vector.tensor_add(tz[0:B, 0:512], uf5[:], bcs(BC_DB1, 512))
            nc.vector.tensor_add(tz[0:B, 512:F], uf2[:],
                                 bc_sb[0:B, BC_DB1 + 512:BC_DB1 + F])
            nc.vector.tensor_mul(z1_bt[0:B, :], tz[0:B, :], gp_bt[0:B, :])

            z1_tr = wrk.tile([128, FO * B], F32R)
            for fo in range(FO):
                tp = pss.tile([128, 32], F32, name="ps", tag="ps")
                nc.tensor.transpose(tp[:],
                                    z1_bt[0:32, fo * 128:(fo + 1) * 128], eye)
                nc.vector.tensor_copy(z1_tr[:, fo * B:(fo + 1) * B],
                                      tp[:, 0:B])

            o5 = pss.tile([B, 512], F32, name="ps", tag="ps")
            o2 = pss.tile([B, 256], F32, name="ps", tag="ps")
            for k in range(KF):
                nc.tensor.matmul(o5[:], z1_tr[:, k * B:(k + 1) * B],
                                 w2r_sb[:, k * F:k * F + 512],
                                 start=(k == 0), stop=(k == KF - 1))
                nc.tensor.matmul(o2[:], z1_tr[:, k * B:(k + 1) * B],
                                 w2r_sb[:, k * F + 512:(k + 1) * F],
                                 start=(k == 0), stop=(k == KF - 1))
            out_bt = wrk.tile([32, F], F32)
            nc.vector.tensor_add(out_bt[0:B, 0:512], o5[:], v_bt[0:B, 0:512])
            nc.vector.tensor_add(out_bt[0:B, 512:F], o2[:], v_bt[0:B, 512:F])
            out2 = wrk.tile([32, F], F32)
            nc.vector.tensor_scalar(out2[0:B, :], out_bt[0:B, :], csel[:],
                                    None, OP.mult)
            nc.scalar.dma_start(OT.ap(), out2[0:B, :])

    nc.compile()
    return nc


def _get_nc():
    if "nc" not in _CACHE:
        _CACHE["nc"] = build()
    return _CACHE["nc"]


def _prep_in_maps(x, W1, b1, W2, b2, mW1, mb1, mW2, mb2, dW1, db1, dW2, db2):
    f32 = np.float32
    bf16 = ml_dtypes.bfloat16
    X = np.ascontiguousarray(np.asarray(x, f32).reshape(B, D))
    XT = np.ascontiguousarray(X.T)
    # packed [128, KD*B]: col n*B+b <- XT[n*128+p, b]
    XTb = np.ascontiguousarray(
        XT.reshape(KD, 128, B).transpose(1, 0, 2).reshape(128, KD * B)
    ).astype(bf16)
    W1 = np.asarray(W1, f32)
    W2c = np.ascontiguousarray(np.asarray(W2, f32))
    mw1t = np.ascontiguousarray(np.asarray(mW1, f32).T)
    mw2t = np.ascontiguousarray(np.asarray(mW2, f32).T)
    b1 = np.asarray(b1, f32); b2 = np.asarray(b2, f32)
    mb1 = np.asarray(mb1, f32); mb2 = np.asarray(mb2, f32)
    dW1 = np.asarray(dW1, f32); db1 = np.asarray(db1, f32)
    dW2 = np.asarray(dW2, f32); db2 = np.asarray(db2, f32)

    fsel = np.zeros((128, B), np.float32)
    for g in range(4):
        for m in range(B):
            fsel[32 * g + m, m] = 1.0
    in_maps = []
    for c in range(NCORES):
        bc = np.zeros((32, BC_W), f32)
        bc[0:B, BC_B1:BC_B1 + F] = b1[None, :]
        bc[0:B, BC_B2:BC_B2 + F] = b2[None, :]
        bc[0:B, BC_DB1:BC_DB1 + F] = db1[c][None, :]
        bc[0:B, BC_DB2:BC_DB2 + F] = db2[c][None, :]
        bc[0:B, BC_MB1:BC_MB1 + HID] = mb1[None, :]
        bc[0:B, BC_MB2:BC_MB2 + T] = mb2[None, :]
        bc[0:B, BC_SEL + c] = 1.0
        bc[0:32, BC_EYE:BC_EYE + 32] = np.eye(32, dtype=f32)
        in_maps.append({
            "xt": XTb,
            "xts": np.ascontiguousarray(
                XT[c * DSH:(c + 1) * DSH, :].reshape(KSH, 128, B)
                .transpose(1, 0, 2).reshape(128, KSH * B)),
            "w1s": np.ascontiguousarray(W1[c * DSH:(c + 1) * DSH, :]),
            "w2": W2c,
            "dw1": np.ascontiguousarray(
                dW1[c].reshape(KJ, 2, 128, F).transpose(0, 2, 1, 3)
                .reshape(KJ * 128, 2 * F)).astype(bf16),
            "dw2": np.ascontiguousarray(dW2[c]),
            "mw1t": mw1t,
            "mw2t": mw2t,
            "bc": bc,
            "fs": fsel,
        })
    return in_maps


def run(inputs, trace=False, trace_cores=None, tmpdir=None):
    nc = _get_nc()
    in_maps = _prep_in_maps(**inputs)
    res = bass_utils.run_bass_kernel_spmd(
        nc, in_maps, core_ids=list(range(NCORES)), trace=trace,
        trace_cores=trace_cores, tmpdir=tmpdir)
    acc = res.results[0]["ft"].astype(np.float64)
    for c in range(NCORES):
        acc = acc + res.results[c]["ot"].astype(np.float64)
    return acc.astype(np.float32), res


def kernel(**inputs):
    out, _ = run(inputs, trace=False)
    return out



# revision 13
# speedup vs baseline: 1.3952x; 1.3952x over previous
"""Trainium2 Bass kernel for nn_MetaNetLinearizedModel (collective-free).

Math (B=16, D=12288, F=768, HID=192, T=8):
    X = x.reshape(B, D)
    h1 = X @ W1 + b1                       [B, F]
    g  = gelu_tanh(h1); gp = gelu_tanh'(h1)
    feats = g @ W2 + b2                    [B, F]
    mh = relu(feats @ mW1.T + mb1)         [B, HID]
    coefs = mh @ mW2.T + mb2               [B, T]
    dh1_t = X @ dW1[t] + db1[t]
    dout  = sum_t coefs[:,t] * ((gp * dh1_t) @ W2 + g @ dW2[t] + db2[t])
    out   = feats + dout

Per-core plan (8 cores, NO collectives -> no cc barrier / dead window):
  - every core streams the FULL W1 in bf16 (18.9 MB) -> h1/g/gp/feats/
    coefs computed locally at bf16 accuracy (this is the critical path).
  - delta path T-sharded: core c owns task t=c. dW1[c] streams in fp8
    (9.4 MB, x64 pre-scale) with DoubleRow perf mode. To keep U = X@dW1
    accurate, x is split 2-term: Xq = fp8(x), Xr = fp8(32*(x - Xq));
    U = (Xq @ dW1q)/64 + (Xr @ dW1q)/2048. Both chains share the same
    moving fp8 tiles.
  - W2 / dW2 / meta-net / stationary casts all bf16.
  - out_c = (c==0)*feats + coefs[:,c]*(z1 @ W2 + g @ dW2[c] + db2[c]),
    z1 = gp*(U + db1[c]); host sums the 8 core outputs.
"""
import sys

sys.path.insert(0, "/opt/trn_rl_repo")

import numpy as np
import ml_dtypes
import concourse.bass as bass
import concourse.bacc as bacc
import concourse.tile as tile
import concourse.mybir as mybir
from concourse import bass_utils

F32 = mybir.dt.float32
BF16 = mybir.dt.bfloat16
FP8 = mybir.dt.float8e4
AF = mybir.ActivationFunctionType
OP = mybir.AluOpType
DR = mybir.MatmulPerfMode.DoubleRow

B = 16
D = 3 * 64 * 64        # 12288
F = 768
HID = 192
T = 8
NCORES = 8
KD = D // 128          # 96 k-tiles over D
KJ = KD // 2           # 48 pair-tiles
KF = F // 128          # 6
WS = 64.0              # fp8 dW1 pre-scale
XRS = 32.0             # fp8 x-residual pre-scale
GELU_C0 = float(np.sqrt(2.0 / np.pi))
GELU_A = 0.044715

# bc pack column offsets (fp32 [32, BCW])
BC_B1 = 0                   # width F
BC_B2 = F                   # width F
BC_DB1 = 2 * F              # width F;  db1[c]
BC_DB2 = 3 * F              # width F;  db2[c]
BC_MB1 = 4 * F              # width HID
BC_MB2 = BC_MB1 + HID       # width T
BC_SEL = BC_MB2 + T         # width T
BC_FB = BC_SEL + T          # width 1; 1.0 on core 0 else 0.0
BC_EYE = BC_FB + 1          # width 32
BCW = BC_EYE + 32

_CACHE = {}


def build():
    nc = bacc.Bacc("TRN2", target_bir_lowering=False, debug=False,
                   enable_asserts=False, num_devices=NCORES)

    XTB = nc.dram_tensor("xtb", [128, KD, B], BF16, kind="ExternalInput")
    XTQ = nc.dram_tensor("xtq", [128, KD, B], FP8, kind="ExternalInput")
    XTR = nc.dram_tensor("xtr", [128, KD, B], FP8, kind="ExternalInput")
    W1B = nc.dram_tensor("w1b", [KJ * 128, 2, F], BF16, kind="ExternalInput")
    DW1Q = nc.dram_tensor("dw1q", [KJ * 128, 2, F], FP8, kind="ExternalInput")
    W2B = nc.dram_tensor("w2b", [F, F], BF16, kind="ExternalInput")
    DW2B = nc.dram_tensor("dw2b", [F, F], BF16, kind="ExternalInput")
    MW1T = nc.dram_tensor("mw1t", [128, KF * HID], BF16, kind="ExternalInput")
    MW2T = nc.dram_tensor("mw2t", [128, 2 * T], BF16, kind="ExternalInput")
    BC = nc.dram_tensor("bc", [32, BCW], F32, kind="ExternalInput")
    OT = nc.dram_tensor("ot", [B, F], F32, kind="ExternalOutput")

    with tile.TileContext(nc, num_cores=NCORES) as tc:
        with (
            tc.tile_pool(name="cst", bufs=1) as cst,
            tc.tile_pool(name="wrk", bufs=1) as wrk,
            tc.tile_pool(name="gtmp", bufs=4) as gtmp,
            tc.tile_pool(name="w1r", bufs=12) as w1r,
            tc.tile_pool(name="dw1r", bufs=12) as dw1r,
            tc.tile_pool(name="psu", bufs=1, space="PSUM") as psu,
            tc.tile_pool(name="pss", bufs=2, space="PSUM") as pss,
        ):
            # ---- priority loads (scalar queue) ----
            xtb_sb = cst.tile([128, KD, B], BF16)
            nc.scalar.dma_start(xtb_sb[:], XTB.ap())
            xtq_sb = cst.tile([128, KD, B], FP8)
            nc.scalar.dma_start(xtq_sb[:], XTQ.ap())
            xtr_sb = cst.tile([128, KD, B], FP8)
            nc.scalar.dma_start(xtr_sb[:], XTR.ap())
            bc_sb = cst.tile([32, BCW], F32)
            nc.scalar.dma_start(bc_sb[:], BC.ap())
            # tail loads (needed only after the W1 stream completes)
            w2b_sb = cst.tile([128, KF * F], BF16)
            for k in range(KF):
                nc.scalar.dma_start(w2b_sb[:, k * F:(k + 1) * F],
                                    W2B.ap()[k * 128:(k + 1) * 128, :])
            mw1t_sb = cst.tile([128, KF * HID], BF16)
            nc.scalar.dma_start(mw1t_sb[:], MW1T.ap())
            mw2t_sb = cst.tile([128, 2 * T], BF16)
            nc.scalar.dma_start(mw2t_sb[:], MW2T.ap())
            dw2b_sb = cst.tile([128, KF * F], BF16)
            for k in range(KF):
                nc.scalar.dma_start(dw2b_sb[:, k * F:(k + 1) * F],
                                    DW2B.ap()[k * 128:(k + 1) * 128, :])

            def bcs(col, w):
                return bc_sb[0:B, col:col + w]
            eye = bc_sb[0:32, BC_EYE:BC_EYE + 32]

            # ---- main stream ----
            # h1 (bf16): h5/h2.  U (fp8 DoubleRow, 2-term x): u5/ur5 + u22
            h5 = psu.tile([B, 512], F32, name="h5")
            h2 = psu.tile([B, 256], F32, name="h2")
            u5 = psu.tile([B, 512], F32, name="u5")
            ur5 = psu.tile([B, 512], F32, name="ur5")
            u2 = psu.tile([B, 256], F32, name="u2")
            ur2 = psu.tile([B, 256], F32, name="ur2")
            for j in range(KJ):
                wt = w1r.tile([128, 2, F], BF16, name="w1t", tag="w1t")
                nc.sync.dma_start(wt[:], W1B.ap()[j * 128:(j + 1) * 128])
                dt_ = dw1r.tile([128, 2, F], FP8, name="dw1t", tag="dw1t")
                nc.gpsimd.dma_start(dt_[:], DW1Q.ap()[j * 128:(j + 1) * 128])
                for h in range(2):
                    st = xtb_sb[:, 2 * j + h, :]
                    nc.tensor.matmul(h5[:], st, wt[:, h, 0:512],
                                     start=(j == 0 and h == 0),
                                     stop=(j == KJ - 1 and h == 1),
                                     skip_group_check=True)
                    nc.tensor.matmul(h2[:], st, wt[:, h, 512:F],
                                     start=(j == 0 and h == 0),
                                     stop=(j == KJ - 1 and h == 1),
                                     skip_group_check=True)
                sq = xtq_sb[:, 2 * j:2 * j + 2, :]
                sr = xtr_sb[:, 2 * j:2 * j + 2, :]
                nc.tensor.matmul(u5[:], sq, dt_[:, :, 0:512], perf_mode=DR,
                                 start=(j == 0), stop=(j == KJ - 1),
                                 skip_group_check=True)
                nc.tensor.matmul(u2[:], sq, dt_[:, :, 512:F],
                                 perf_mode=DR,
                                 start=(j == 0), stop=(j == KJ - 1),
                                 skip_group_check=True)
                nc.tensor.matmul(ur5[:], sr, dt_[:, :, 0:512], perf_mode=DR,
                                 start=(j == 0), stop=(j == KJ - 1),
                                 skip_group_check=True)
                nc.tensor.matmul(ur2[:], sr, dt_[:, :, 512:F],
                                 perf_mode=DR,
                                 start=(j == 0), stop=(j == KJ - 1),
                                 skip_group_check=True)

            # ---- h1 = psum + b1 ; gelu chain -> g, gp ----
            h1c = wrk.tile([32, F], F32)
            nc.vector.memset(h1c[:], 0.0)
            nc.vector.tensor_add(h1c[0:B, 0:512], h5[:], bcs(BC_B1, 512))
            nc.vector.tensor_add(h1c[0:B, 512:F], h2[:],
                                 bc_sb[0:B, BC_B1 + 512:BC_B1 + F])

            def gt():
                return gtmp.tile([32, F], F32, name="gt", tag="gt")
            h1 = h1c[0:B, :]
            s_ = gt(); nc.vector.tensor_mul(s_[0:B, :], h1, h1)
            t1 = gt(); nc.vector.scalar_tensor_tensor(
                t1[0:B, :], s_[0:B, :], GELU_A, h1, OP.mult, OP.mult)
            up = gt(); nc.vector.tensor_add(up[0:B, :], h1, t1[0:B, :])
            t_sb = wrk.tile([32, F], F32)
            nc.scalar.activation(t_sb[0:B, :], up[0:B, :], AF.Tanh,
                                 scale=GELU_C0)
            tt = t_sb[0:B, :]
            th_sb = wrk.tile([32, F], F32)
            nc.vector.tensor_scalar(th_sb[0:B, :], tt, 0.5, 0.5,
                                    OP.mult, OP.add)
            g_c = wrk.tile([32, F], F32)
            nc.vector.memset(g_c[:], 0.0)
            nc.vector.tensor_mul(g_c[0:B, :], th_sb[0:B, :], h1)
            t2 = gt(); nc.vector.tensor_mul(t2[0:B, :], tt, tt)
            q_ = gt(); nc.vector.tensor_scalar(q_[0:B, :], t2[0:B, :],
                                               -1.0, 1.0, OP.mult, OP.add)
            r_ = gt(); nc.vector.tensor_scalar(r_[0:B, :], s_[0:B, :],
                                               3.0 * GELU_A, 1.0,
                                               OP.mult, OP.add)
            m_ = gt(); nc.vector.tensor_mul(m_[0:B, :], q_[0:B, :], r_[0:B, :])
            n_ = gt(); nc.vector.tensor_mul(n_[0:B, :], m_[0:B, :], h1)
            gp_bt = wrk.tile([32, F], F32)
            nc.vector.scalar_tensor_tensor(
                gp_bt[0:B, :], n_[0:B, :], 0.5 * GELU_C0, th_sb[0:B, :],
                OP.mult, OP.add)

            # ---- transpose g -> bf16 stationary [f, b] ----
            gT_b = wrk.tile([128, KF * B], BF16)
            for fo in range(KF):
                tp = pss.tile([128, 32], F32, name="ps", tag="ps")
                nc.tensor.transpose(tp[:], g_c[0:32, fo * 128:(fo + 1) * 128],
                                    eye)
                nc.vector.tensor_copy(gT_b[:, fo * B:(fo + 1) * B],
                                      tp[:, 0:B])

            # ---- feats = g @ W2 + b2 ----
            f5 = pss.tile([B, 512], F32, name="ps", tag="ps")
            f2 = pss.tile([B, 256], F32, name="ps", tag="ps")
            for k in range(KF):
                nc.tensor.matmul(f5[:], gT_b[:, k * B:(k + 1) * B],
                                 w2b_sb[:, k * F:k * F + 512],
                                 start=(k == 0), stop=(k == KF - 1),
                                 skip_group_check=True)
                nc.tensor.matmul(f2[:], gT_b[:, k * B:(k + 1) * B],
                                 w2b_sb[:, k * F + 512:(k + 1) * F],
                                 start=(k == 0), stop=(k == KF - 1),
                                 skip_group_check=True)
            feats = wrk.tile([32, F], F32)
            nc.vector.memset(feats[:], 0.0)
            nc.vector.tensor_add(feats[0:B, 0:512], f5[:], bcs(BC_B2, 512))
            nc.vector.tensor_add(feats[0:B, 512:F], f2[:],
                                 bc_sb[0:B, BC_B2 + 512:BC_B2 + F])

            featsT = wrk.tile([128, KF * B], BF16)
            for fo in range(KF):
                tp = pss.tile([128, 32], F32, name="ps", tag="ps")
                nc.tensor.transpose(tp[:],
                                    feats[0:32, fo * 128:(fo + 1) * 128], eye)
                nc.vector.tensor_copy(featsT[:, fo * B:(fo + 1) * B],
                                      tp[:, 0:B])

            # ---- meta-net -> coefs -> csel ----
            mps = pss.tile([B, HID], F32, name="ps", tag="ps")
            for k in range(KF):
                nc.tensor.matmul(mps[:], featsT[:, k * B:(k + 1) * B],
                                 mw1t_sb[:, k * HID:(k + 1) * HID],
                                 start=(k == 0), stop=(k == KF - 1),
                                 skip_group_check=True)
            mh_bt = wrk.tile([32, HID], F32)
            nc.vector.memset(mh_bt[:], 0.0)
            mtmp = gtmp.tile([32, F], F32, name="gt", tag="gt")
            nc.vector.tensor_add(mtmp[0:B, 0:HID], mps[:], bcs(BC_MB1, HID))
            nc.vector.tensor_relu(mh_bt[0:B, :], mtmp[0:B, 0:HID])

            mh_tr = wrk.tile([128, 2 * B], BF16)
            tp = pss.tile([128, 32], F32, name="ps", tag="ps")
            nc.tensor.transpose(tp[:], mh_bt[0:32, 0:128], eye)
            nc.vector.tensor_copy(mh_tr[:, 0:B], tp[:, 0:B])
            tp = pss.tile([128, 32], F32, name="ps", tag="ps")
            nc.tensor.transpose(tp[0:HID - 128, :], mh_bt[0:32, 128:HID], eye)
            nc.vector.tensor_copy(mh_tr[0:HID - 128, B:2 * B],
                                  tp[0:HID - 128, 0:B])

            cps = pss.tile([B, T], F32, name="ps", tag="ps")
            nc.tensor.matmul(cps[:], mh_tr[:, 0:B], mw2t_sb[:, 0:T],
                             start=True, stop=False, skip_group_check=True)
            nc.tensor.matmul(cps[:], mh_tr[0:HID - 128, B:2 * B],
                             mw2t_sb[0:HID - 128, T:2 * T],
                             start=False, stop=True, skip_group_check=True)
            coefs_bt = wrk.tile([B, T], F32)
            nc.vector.tensor_add(coefs_bt[:], cps[:], bcs(BC_MB2, T))
            csel = wrk.tile([B, 1], F32)
            cjunk = wrk.tile([B, T], F32)
            nc.vector.tensor_mul(cjunk[:], coefs_bt[:], bcs(BC_SEL, T))
            nc.vector.reduce_sum(csel[:], cjunk[:], axis=mybir.AxisListType.X)

            # ---- V = g @ dW2 + db2 (bf16) ----
            v5 = pss.tile([B, 512], F32, name="ps", tag="ps")
            v2 = pss.tile([B, 256], F32, name="ps", tag="ps")
            for k in range(KF):
                nc.tensor.matmul(v5[:], gT_b[:, k * B:(k + 1) * B],
                                 dw2b_sb[:, k * F:k * F + 512],
                                 start=(k == 0), stop=(k == KF - 1),
                                 skip_group_check=True)
                nc.tensor.matmul(v2[:], gT_b[:, k * B:(k + 1) * B],
                                 dw2b_sb[:, k * F + 512:(k + 1) * F],
                                 start=(k == 0), stop=(k == KF - 1),
                                 skip_group_check=True)
            v_bt = wrk.tile([32, F], F32)
            nc.vector.tensor_add(v_bt[0:B, 0:512], v5[:], bcs(BC_DB2, 512))
            nc.vector.tensor_add(v_bt[0:B, 512:F], v2[:],
                                 bc_sb[0:B, BC_DB2 + 512:BC_DB2 + F])

            # ---- z1 = gp * (U + db1);  U = u/WS + ur/(WS*XRS) ----
            z0 = gtmp.tile([32, F], F32, name="gt", tag="gt")
            nc.vector.scalar_tensor_tensor(
                z0[0:B, 0:512], u5[:], 1.0 / WS,
                bc_sb[0:B, BC_DB1:BC_DB1 + 512], OP.mult, OP.add)
            nc.vector.scalar_tensor_tensor(
                z0[0:B, 512:F], u2[:], 1.0 / WS,
                bc_sb[0:B, BC_DB1 + 512:BC_DB1 + F], OP.mult, OP.add)
            z0b = gtmp.tile([32, F], F32, name="gt", tag="gt")
            nc.vector.scalar_tensor_tensor(
                z0b[0:B, 0:512], ur5[:], 1.0 / (WS * XRS),
                z0[0:B, 0:512], OP.mult, OP.add)
            nc.vector.scalar_tensor_tensor(
                z0b[0:B, 512:F], ur2[:], 1.0 / (WS * XRS),
                z0[0:B, 512:F], OP.mult, OP.add)
            z1c = wrk.tile([32, F], F32)
            nc.vector.memset(z1c[:], 0.0)
            nc.vector.tensor_mul(z1c[0:B, :], z0b[0:B, :], gp_bt[0:B, :])

            z1T = wrk.tile([128, KF * B], BF16)
            for fo in range(KF):
                tp = pss.tile([128, 32], F32, name="ps", tag="ps")
                nc.tensor.transpose(tp[:],
                                    z1c[0:32, fo * 128:(fo + 1) * 128], eye)
                nc.vector.tensor_copy(z1T[:, fo * B:(fo + 1) * B],
                                      tp[:, 0:B])

            # ---- dout1 = z1 @ W2 ; ot = fb*feats + csel*(dout1 + V) ----
            o5 = pss.tile([B, 512], F32, name="ps", tag="ps")
            o2 = pss.tile([B, 256], F32, name="ps", tag="ps")
            for k in range(KF):
                nc.tensor.matmul(o5[:], z1T[:, k * B:(k + 1) * B],
                                 w2b_sb[:, k * F:k * F + 512],
                                 start=(k == 0), stop=(k == KF - 1),
                                 skip_group_check=True)
                nc.tensor.matmul(o2[:], z1T[:, k * B:(k + 1) * B],
                                 w2b_sb[:, k * F + 512:(k + 1) * F],
                                 start=(k == 0), stop=(k == KF - 1),
                                 skip_group_check=True)
            od = wrk.tile([32, F], F32)
            nc.vector.tensor_add(od[0:B, 0:512], o5[:], v_bt[0:B, 0:512])
            nc.vector.tensor_add(od[0:B, 512:F], o2[:], v_bt[0:B, 512:F])
            od2 = wrk.tile([32, F], F32)
            nc.vector.tensor_scalar(od2[0:B, :], od[0:B, :], csel[:],
                                    None, OP.mult)
            fmask = wrk.tile([32, F], F32)
            nc.vector.tensor_scalar(fmask[0:B, :], feats[0:B, :],
                                    bc_sb[0:B, BC_FB:BC_FB + 1],
                                    None, OP.mult)
            out2 = wrk.tile([32, F], F32)
            nc.vector.tensor_add(out2[0:B, :], od2[0:B, :], fmask[0:B, :])
            nc.scalar.dma_start(OT.ap(), out2[0:B, :])

    nc.compile()
    return nc


def _get_nc():
    if "nc" not in _CACHE:
        _CACHE["nc"] = build()
    return _CACHE["nc"]


def _prep_in_maps(x, W1, b1, W2, b2, mW1, mb1, mW2, mb2, dW1, db1, dW2, db2):
    f32 = np.float32
    bf16 = ml_dtypes.bfloat16
    fp8 = ml_dtypes.float8_e4m3
    X = np.ascontiguousarray(np.asarray(x, f32).reshape(B, D))
    XT = np.ascontiguousarray(X.T)                       # [D, B]
    xt3 = np.ascontiguousarray(
        XT.reshape(KD, 128, B).transpose(1, 0, 2))       # [128, KD, B]
    xtb = xt3.astype(bf16)
    xtq = xt3.astype(fp8)
    xtr = ((xt3 - xtq.astype(f32)) * XRS).astype(fp8)
    W1 = np.asarray(W1, f32)
    W2 = np.asarray(W2, f32)
    b1 = np.asarray(b1, f32); b2 = np.asarray(b2, f32)
    mb1 = np.asarray(mb1, f32); mb2 = np.asarray(mb2, f32)
    dW1 = np.asarray(dW1, f32); db1 = np.asarray(db1, f32)
    dW2 = np.asarray(dW2, f32); db2 = np.asarray(db2, f32)

    def pairs(w):  # [D, F] -> [KJ*128, 2, F]
        return np.ascontiguousarray(
            w.reshape(KJ, 2, 128, F).transpose(0, 2, 1, 3)
            .reshape(KJ * 128, 2, F))

    w1b = pairs(W1).astype(bf16)
    w2b = np.ascontiguousarray(W2).astype(bf16)
    mw1t = np.ascontiguousarray(
        np.asarray(mW1, f32).T.reshape(KF, 128, HID).transpose(1, 0, 2)
        .reshape(128, KF * HID)).astype(bf16)
    mw2tf = np.asarray(mW2, f32).T                       # [HID, T]
    mw2t = np.zeros((128, 2 * T), f32)
    mw2t[:, 0:T] = mw2tf[0:128, :]
    mw2t[0:HID - 128, T:2 * T] = mw2tf[128:HID, :]
    mw2t = mw2t.astype(bf16)

    in_maps = []
    for c in range(NCORES):
        bc = np.zeros((32, BCW), f32)
        bc[0:B, BC_B1:BC_B1 + F] = b1[None, :]
        bc[0:B, BC_B2:BC_B2 + F] = b2[None, :]
        bc[0:B, BC_DB1:BC_DB1 + F] = db1[c][None, :]
        bc[0:B, BC_DB2:BC_DB2 + F] = db2[c][None, :]
        bc[0:B, BC_MB1:BC_MB1 + HID] = mb1[None, :]
        bc[0:B, BC_MB2:BC_MB2 + T] = mb2[None, :]
        bc[0:B, BC_SEL + c] = 1.0
        if c == 0:
            bc[0:B, BC_FB] = 1.0
        bc[0:32, BC_EYE:BC_EYE + 32] = np.eye(32, dtype=f32)
        in_maps.append({
            "xtb": xtb,
            "xtq": xtq,
            "xtr": xtr,
            "w1b": w1b,
            "dw1q": (pairs(dW1[c]) * WS).astype(fp8),
            "w2b": w2b,
            "dw2b": np.ascontiguousarray(dW2[c]).astype(bf16),
            "mw1t": mw1t,
            "mw2t": mw2t,
            "bc": bc,
        })
    return in_maps


def run(inputs, trace=False, trace_cores=None, tmpdir=None):
    nc = _get_nc()
    in_maps = _prep_in_maps(**inputs)
    res = bass_utils.run_bass_kernel_spmd(
        nc, in_maps, core_ids=list(range(NCORES)), trace=trace,
        trace_cores=trace_cores, tmpdir=tmpdir)
    acc = res.results[0]["ot"].astype(np.float64)
    for c in range(1, NCORES):
        acc = acc + res.results[c]["ot"].astype(np.float64)
    return acc.astype(np.float32), res


def kernel(**inputs):
    out, _ = run(inputs, trace=False)
    return out


# revision 14
# speedup vs baseline: 1.4188x; 1.0170x over previous
"""Trainium2 Bass kernel for nn_MetaNetLinearizedModel (collective-free).

Math (B=16, D=12288, F=768, HID=192, T=8):
    X = x.reshape(B, D)
    h1 = X @ W1 + b1                       [B, F]
    g  = gelu_tanh(h1); gp = gelu_tanh'(h1)
    feats = g @ W2 + b2                    [B, F]
    mh = relu(feats @ mW1.T + mb1)         [B, HID]
    coefs = mh @ mW2.T + mb2               [B, T]
    dh1_t = X @ dW1[t] + db1[t]
    dout  = sum_t coefs[:,t] * ((gp * dh1_t) @ W2 + g @ dW2[t] + db2[t])
    out   = feats + dout

Per-core plan (8 cores, NO collectives -> no cc barrier / dead window):
  - every core streams the FULL W1 in bf16 (18.9 MB) -> h1/g/gp/feats/
    coefs computed locally at bf16 accuracy (this is the critical path).
  - delta path T-sharded: core c owns task t=c. dW1[c] streams in fp8
    (9.4 MB, x64 pre-scale) with DoubleRow perf mode. To keep U = X@dW1
    accurate, x is split 2-term: Xq = fp8(x), Xr = fp8(32*(x - Xq));
    U = (Xq @ dW1q)/64 + (Xr @ dW1q)/2048. Both chains share the same
    moving fp8 tiles.
  - W2 / dW2 / meta-net / stationary casts all bf16.
  - out_c = (c==0)*feats + coefs[:,c]*(z1 @ W2 + g @ dW2[c] + db2[c]),
    z1 = gp*(U + db1[c]); host sums the 8 core outputs.
"""
import sys

sys.path.insert(0, "/opt/trn_rl_repo")

import numpy as np
import ml_dtypes
import concourse.bass as bass
import concourse.bacc as bacc
import concourse.tile as tile
import concourse.mybir as mybir
from concourse import bass_utils

F32 = mybir.dt.float32
BF16 = mybir.dt.bfloat16
FP8 = mybir.dt.float8e4
AF = mybir.ActivationFunctionType
OP = mybir.AluOpType
DR = mybir.MatmulPerfMode.DoubleRow

B = 16
D = 3 * 64 * 64        # 12288
F = 768
HID = 192
T = 8
NCORES = 8
KD = D // 128          # 96 k-tiles over D
KJ = KD // 2           # 48 pair-tiles
KF = F // 128          # 6
WS = 64.0              # fp8 dW1 pre-scale
XRS = 32.0             # fp8 x-residual pre-scale
GELU_C0 = float(np.sqrt(2.0 / np.pi))
GELU_A = 0.044715

# bc pack column offsets (fp32 [32, BCW])
BC_B1 = 0                   # width F
BC_B2 = F                   # width F
BC_DB1 = 2 * F              # width F;  db1[c]
BC_DB2 = 3 * F              # width F;  db2[c]
BC_MB1 = 4 * F              # width HID
BC_MB2 = BC_MB1 + HID       # width T
BC_SEL = BC_MB2 + T         # width T
BC_FB = BC_SEL + T          # width 1; 1.0 on core 0 else 0.0
BC_EYE = BC_FB + 1          # width 32
BCW = BC_EYE + 32

_CACHE = {}


def build():
    nc = bacc.Bacc("TRN2", target_bir_lowering=False, debug=False,
                   enable_asserts=False, num_devices=NCORES)

    XTB = nc.dram_tensor("xtb", [128, KD, B], BF16, kind="ExternalInput")
    XTQ = nc.dram_tensor("xtq", [128, KD, B], FP8, kind="ExternalInput")
    XTR = nc.dram_tensor("xtr", [128, KD, B], FP8, kind="ExternalInput")
    W1B = nc.dram_tensor("w1b", [KJ * 128, 2, F], BF16, kind="ExternalInput")
    DW1Q = nc.dram_tensor("dw1q", [KJ * 128, 2, F], FP8, kind="ExternalInput")
    W2B = nc.dram_tensor("w2b", [F, F], BF16, kind="ExternalInput")
    DW2B = nc.dram_tensor("dw2b", [F, F], BF16, kind="ExternalInput")
    MW1T = nc.dram_tensor("mw1t", [128, KF * HID], BF16, kind="ExternalInput")
    MW2T = nc.dram_tensor("mw2t", [128, 2 * T], BF16, kind="ExternalInput")
    BC = nc.dram_tensor("bc", [32, BCW], F32, kind="ExternalInput")
    OT = nc.dram_tensor("ot", [B, F], F32, kind="ExternalOutput")

    with tile.TileContext(nc, num_cores=NCORES) as tc:
        with (
            tc.tile_pool(name="cst", bufs=1) as cst,
            tc.tile_pool(name="wrk", bufs=1) as wrk,
            tc.tile_pool(name="gtmp", bufs=4) as gtmp,
            tc.tile_pool(name="w1r", bufs=20) as w1r,
            tc.tile_pool(name="dw1r", bufs=16) as dw1r,
            tc.tile_pool(name="psu", bufs=1, space="PSUM") as psu,
            tc.tile_pool(name="pss", bufs=2, space="PSUM") as pss,
        ):
            # ---- priority loads (scalar queue) ----
            xtb_sb = cst.tile([128, KD, B], BF16)
            nc.scalar.dma_start(xtb_sb[:], XTB.ap())
            xtq_sb = cst.tile([128, KD, B], FP8)
            nc.scalar.dma_start(xtq_sb[:], XTQ.ap())
            xtr_sb = cst.tile([128, KD, B], FP8)
            nc.scalar.dma_start(xtr_sb[:], XTR.ap())
            bc_sb = cst.tile([32, BCW], F32)
            nc.scalar.dma_start(bc_sb[:], BC.ap())
            # tail loads (needed only after the W1 stream completes)
            w2b_sb = cst.tile([128, KF * F], BF16)
            for k in range(KF):
                nc.scalar.dma_start(w2b_sb[:, k * F:(k + 1) * F],
                                    W2B.ap()[k * 128:(k + 1) * 128, :])
            mw1t_sb = cst.tile([128, KF * HID], BF16)
            nc.scalar.dma_start(mw1t_sb[:], MW1T.ap())
            mw2t_sb = cst.tile([128, 2 * T], BF16)
            nc.scalar.dma_start(mw2t_sb[:], MW2T.ap())
            dw2b_sb = cst.tile([128, KF * F], BF16)
            for k in range(KF):
                nc.scalar.dma_start(dw2b_sb[:, k * F:(k + 1) * F],
                                    DW2B.ap()[k * 128:(k + 1) * 128, :])

            scr = wrk.tile([1, 8], F32)
            nc.scalar.activation(scr[0:1, 0:1], bc_sb[0:1, 0:1],
                                 AF.Gelu_apprx_tanh)

            def bcs(col, w):
                return bc_sb[0:B, col:col + w]
            eye = bc_sb[0:32, BC_EYE:BC_EYE + 32]

            # ---- main stream ----
            # h1 (bf16): h5/h2.  U (fp8 DoubleRow, 2-term x): u5/ur5 + u22
            h5 = psu.tile([B, 512], F32, name="h5")
            h2 = psu.tile([B, 256], F32, name="h2")
            u5 = psu.tile([B, 512], F32, name="u5")
            ur5 = psu.tile([B, 512], F32, name="ur5")
            u2 = psu.tile([B, 256], F32, name="u2")
            ur2 = psu.tile([B, 256], F32, name="ur2")
            for j in range(KJ):
                wt = w1r.tile([128, 2, F], BF16, name="w1t", tag="w1t")
                ew, ed = (nc.sync, nc.gpsimd) if j % 2 == 0 else \
                         (nc.gpsimd, nc.sync)
                ew.dma_start(wt[:], W1B.ap()[j * 128:(j + 1) * 128])
                dt_ = dw1r.tile([128, 2, F], FP8, name="dw1t", tag="dw1t")
                ed.dma_start(dt_[:], DW1Q.ap()[j * 128:(j + 1) * 128])
                for h in range(2):
                    st = xtb_sb[:, 2 * j + h, :]
                    nc.tensor.matmul(h5[:], st, wt[:, h, 0:512],
                                     start=(j == 0 and h == 0),
                                     stop=(j == KJ - 1 and h == 1),
                                     skip_group_check=True)
                    nc.tensor.matmul(h2[:], st, wt[:, h, 512:F],
                                     start=(j == 0 and h == 0),
                                     stop=(j == KJ - 1 and h == 1),
                                     skip_group_check=True)
                sq = xtq_sb[:, 2 * j:2 * j + 2, :]
                sr = xtr_sb[:, 2 * j:2 * j + 2, :]
                nc.tensor.matmul(u5[:], sq, dt_[:, :, 0:512], perf_mode=DR,
                                 start=(j == 0), stop=(j == KJ - 1),
                                 skip_group_check=True)
                nc.tensor.matmul(u2[:], sq, dt_[:, :, 512:F],
                                 perf_mode=DR,
                                 start=(j == 0), stop=(j == KJ - 1),
                                 skip_group_check=True)
                nc.tensor.matmul(ur5[:], sr, dt_[:, :, 0:512], perf_mode=DR,
                                 start=(j == 0), stop=(j == KJ - 1),
                                 skip_group_check=True)
                nc.tensor.matmul(ur2[:], sr, dt_[:, :, 512:F],
                                 perf_mode=DR,
                                 start=(j == 0), stop=(j == KJ - 1),
                                 skip_group_check=True)

            # ---- h1 = psum + b1 ; g/gp via activation LUTs ----
            eye16 = bc_sb[0:B, BC_EYE:BC_EYE + B]
            h1c = wrk.tile([B, F], F32)
            nc.vector.tensor_add(h1c[:, 0:512], h5[:], bcs(BC_B1, 512))
            nc.vector.tensor_add(h1c[:, 512:F], h2[:],
                                 bc_sb[0:B, BC_B1 + 512:BC_B1 + F])
            g_c = wrk.tile([B, F], F32)
            nc.scalar.activation(g_c[:], h1c[:], AF.Gelu_apprx_tanh)

            # ---- transpose g -> bf16 stationary [f, b] ----
            gT_b = wrk.tile([128, KF * B], BF16)
            for fo in range(KF):
                tp = pss.tile([128, B], F32, name="ps", tag="ps")
                nc.tensor.transpose(tp[:], g_c[:, fo * 128:(fo + 1) * 128],
                                    eye16)
                nc.vector.tensor_copy(gT_b[:, fo * B:(fo + 1) * B], tp[:])

            # ---- feats = g @ W2 + b2 ; V = g @ dW2 (independent) ----
            f5 = pss.tile([B, 512], F32, name="ps", tag="ps")
            f2 = pss.tile([B, 256], F32, name="ps", tag="ps")
            for k in range(KF):
                nc.tensor.matmul(f5[:], gT_b[:, k * B:(k + 1) * B],
                                 w2b_sb[:, k * F:k * F + 512],
                                 start=(k == 0), stop=(k == KF - 1),
                                 skip_group_check=True)
                nc.tensor.matmul(f2[:], gT_b[:, k * B:(k + 1) * B],
                                 w2b_sb[:, k * F + 512:(k + 1) * F],
                                 start=(k == 0), stop=(k == KF - 1),
                                 skip_group_check=True)
            v5 = pss.tile([B, 512], F32, name="ps", tag="ps")
            v2 = pss.tile([B, 256], F32, name="ps", tag="ps")
            for k in range(KF):
                nc.tensor.matmul(v5[:], gT_b[:, k * B:(k + 1) * B],
                                 dw2b_sb[:, k * F:k * F + 512],
                                 start=(k == 0), stop=(k == KF - 1),
                                 skip_group_check=True)
                nc.tensor.matmul(v2[:], gT_b[:, k * B:(k + 1) * B],
                                 dw2b_sb[:, k * F + 512:(k + 1) * F],
                                 start=(k == 0), stop=(k == KF - 1),
                                 skip_group_check=True)
            gp_bt = wrk.tile([B, F], F32)
            nc.scalar.activation(gp_bt[:], h1c[:], AF.Derivative_Gelu)

            feats = wrk.tile([B, F], F32)
            nc.vector.tensor_add(feats[:, 0:512], f5[:], bcs(BC_B2, 512))
            nc.vector.tensor_add(feats[:, 512:F], f2[:],
                                 bc_sb[0:B, BC_B2 + 512:BC_B2 + F])
            fmask = wrk.tile([B, F], F32)
            nc.vector.tensor_scalar(fmask[:], feats[:],
                                    bc_sb[0:B, BC_FB:BC_FB + 1],
                                    None, OP.mult)

            featsT = wrk.tile([128, KF * B], BF16)
            for fo in range(KF):
                tp = pss.tile([128, B], F32, name="ps", tag="ps")
                nc.tensor.transpose(tp[:], feats[:, fo * 128:(fo + 1) * 128],
                                    eye16)
                nc.vector.tensor_copy(featsT[:, fo * B:(fo + 1) * B], tp[:])

            # ---- meta-net -> coefs -> csel ----
            mps = pss.tile([B, HID], F32, name="ps", tag="ps")
            for k in range(KF):
                nc.tensor.matmul(mps[:], featsT[:, k * B:(k + 1) * B],
                                 mw1t_sb[:, k * HID:(k + 1) * HID],
                                 start=(k == 0), stop=(k == KF - 1),
                                 skip_group_check=True)
            mh_bt = wrk.tile([B, HID], F32)
            mtmp = gtmp.tile([B, HID], F32, name="mt", tag="mt")
            nc.vector.tensor_add(mtmp[:], mps[:], bcs(BC_MB1, HID))
            nc.vector.tensor_relu(mh_bt[:], mtmp[:])

            mh_tr = wrk.tile([128, 2 * B], BF16)
            tp = pss.tile([128, B], F32, name="ps", tag="ps")
            nc.tensor.transpose(tp[:], mh_bt[:, 0:128], eye16)
            nc.vector.tensor_copy(mh_tr[:, 0:B], tp[:])
            tp = pss.tile([128, B], F32, name="ps", tag="ps")
            nc.tensor.transpose(tp[0:HID - 128, :], mh_bt[:, 128:HID], eye16)
            nc.vector.tensor_copy(mh_tr[0:HID - 128, B:2 * B],
                                  tp[0:HID - 128, :])

            cps = pss.tile([B, T], F32, name="ps", tag="ps")
            nc.tensor.matmul(cps[:], mh_tr[:, 0:B], mw2t_sb[:, 0:T],
                             start=True, stop=False, skip_group_check=True)
            nc.tensor.matmul(cps[:], mh_tr[0:HID - 128, B:2 * B],
                             mw2t_sb[0:HID - 128, T:2 * T],
                             start=False, stop=True, skip_group_check=True)
            coefs_bt = wrk.tile([B, T], F32)
            nc.vector.tensor_add(coefs_bt[:], cps[:], bcs(BC_MB2, T))
            csel = wrk.tile([B, 1], F32)
            cjunk = wrk.tile([B, T], F32)
            nc.vector.tensor_mul(cjunk[:], coefs_bt[:], bcs(BC_SEL, T))
            nc.vector.reduce_sum(csel[:], cjunk[:], axis=mybir.AxisListType.X)

            # ---- V bias ----
            v_bt = wrk.tile([B, F], F32)
            nc.vector.tensor_add(v_bt[:, 0:512], v5[:], bcs(BC_DB2, 512))
            nc.vector.tensor_add(v_bt[:, 512:F], v2[:],
                                 bc_sb[0:B, BC_DB2 + 512:BC_DB2 + F])

            # ---- z1 = gp * (U + db1);  U = u/WS + ur/(WS*XRS) ----
            z0 = gtmp.tile([B, F], F32, name="gt", tag="gt")
            nc.vector.scalar_tensor_tensor(
                z0[:, 0:512], u5[:], 1.0 / WS,
                bc_sb[0:B, BC_DB1:BC_DB1 + 512], OP.mult, OP.add)
            nc.vector.scalar_tensor_tensor(
                z0[:, 512:F], u2[:], 1.0 / WS,
                bc_sb[0:B, BC_DB1 + 512:BC_DB1 + F], OP.mult, OP.add)
            z0b = gtmp.tile([B, F], F32, name="gt", tag="gt")
            nc.vector.scalar_tensor_tensor(
                z0b[:, 0:512], ur5[:], 1.0 / (WS * XRS),
                z0[:, 0:512], OP.mult, OP.add)
            nc.vector.scalar_tensor_tensor(
                z0b[:, 512:F], ur2[:], 1.0 / (WS * XRS),
                z0[:, 512:F], OP.mult, OP.add)
            z1c = wrk.tile([B, F], F32)
            nc.vector.tensor_mul(z1c[:], z0b[:], gp_bt[:])

            z1T = wrk.tile([128, KF * B], BF16)
            for fo in range(KF):
                tp = pss.tile([128, B], F32, name="ps", tag="ps")
                nc.tensor.transpose(tp[:], z1c[:, fo * 128:(fo + 1) * 128],
                                    eye16)
                nc.vector.tensor_copy(z1T[:, fo * B:(fo + 1) * B], tp[:])

            # ---- dout1 = z1 @ W2 ; ot = fb*feats + csel*(dout1 + V) ----
            o5 = pss.tile([B, 512], F32, name="ps", tag="ps")
            o2 = pss.tile([B, 256], F32, name="ps", tag="ps")
            for k in range(KF):
                nc.tensor.matmul(o5[:], z1T[:, k * B:(k + 1) * B],
                                 w2b_sb[:, k * F:k * F + 512],
                                 start=(k == 0), stop=(k == KF - 1),
                                 skip_group_check=True)
                nc.tensor.matmul(o2[:], z1T[:, k * B:(k + 1) * B],
                                 w2b_sb[:, k * F + 512:(k + 1) * F],
                                 start=(k == 0), stop=(k == KF - 1),
                                 skip_group_check=True)
            od = wrk.tile([B, F], F32)
            nc.vector.tensor_add(od[:, 0:512], o5[:], v_bt[:, 0:512])
            nc.vector.tensor_add(od[:, 512:F], o2[:], v_bt[:, 512:F])
            od2 = wrk.tile([B, F], F32)
            nc.vector.tensor_scalar(od2[:], od[:, :], csel[:],
                                    None, OP.mult)
            out2 = wrk.tile([B, F], F32)
            nc.vector.tensor_add(out2[:], od2[:], fmask[:])
            nc.scalar.dma_start(OT.ap(), out2[:])

    nc.compile()
    return nc


def _get_nc():
    if "nc" not in _CACHE:
        _CACHE["nc"] = build()
    return _CACHE["nc"]


def _prep_in_maps(x, W1, b1, W2, b2, mW1, mb1, mW2, mb2, dW1, db1, dW2, db2):
    f32 = np.float32
    bf16 = ml_dtypes.bfloat16
    fp8 = ml_dtypes.float8_e4m3
    X = np.ascontiguousarray(np.asarray(x, f32).reshape(B, D))
    XT = np.ascontiguousarray(X.T)                       # [D, B]
    xt3 = np.ascontiguousarray(
        XT.reshape(KD, 128, B).transpose(1, 0, 2))       # [128, KD, B]
    xtb = xt3.astype(bf16)
    xtq = xt3.astype(fp8)
    xtr = ((xt3 - xtq.astype(f32)) * XRS).astype(fp8)
    W1 = np.asarray(W1, f32)
    W2 = np.asarray(W2, f32)
    b1 = np.asarray(b1, f32); b2 = np.asarray(b2, f32)
    mb1 = np.asarray(mb1, f32); mb2 = np.asarray(mb2, f32)
    dW1 = np.asarray(dW1, f32); db1 = np.asarray(db1, f32)
    dW2 = np.asarray(dW2, f32); db2 = np.asarray(db2, f32)

    def pairs(w):  # [D, F] -> [KJ*128, 2, F]
        return np.ascontiguousarray(
            w.reshape(KJ, 2, 128, F).transpose(0, 2, 1, 3)
            .reshape(KJ * 128, 2, F))

    w1b = pairs(W1).astype(bf16)
    w2b = np.ascontiguousarray(W2).astype(bf16)
    mw1t = np.ascontiguousarray(
        np.asarray(mW1, f32).T.reshape(KF, 128, HID).transpose(1, 0, 2)
        .reshape(128, KF * HID)).astype(bf16)
    mw2tf = np.asarray(mW2, f32).T                       # [HID, T]
    mw2t = np.zeros((128, 2 * T), f32)
    mw2t[:, 0:T] = mw2tf[0:128, :]
    mw2t[0:HID - 128, T:2 * T] = mw2tf[128:HID, :]
    mw2t = mw2t.astype(bf16)

    in_maps = []
    for c in range(NCORES):
        bc = np.zeros((32, BCW), f32)
        bc[0:B, BC_B1:BC_B1 + F] = b1[None, :]
        bc[0:B, BC_B2:BC_B2 + F] = b2[None, :]
        bc[0:B, BC_DB1:BC_DB1 + F] = db1[c][None, :]
        bc[0:B, BC_DB2:BC_DB2 + F] = db2[c][None, :]
        bc[0:B, BC_MB1:BC_MB1 + HID] = mb1[None, :]
        bc[0:B, BC_MB2:BC_MB2 + T] = mb2[None, :]
        bc[0:B, BC_SEL + c] = 1.0
        if c == 0:
            bc[0:B, BC_FB] = 1.0
        bc[0:32, BC_EYE:BC_EYE + 32] = np.eye(32, dtype=f32)
        in_maps.append({
            "xtb": xtb,
            "xtq": xtq,
            "xtr": xtr,
            "w1b": w1b,
            "dw1q": (pairs(dW1[c]) * WS).astype(fp8),
            "w2b": w2b,
            "dw2b": np.ascontiguousarray(dW2[c]).astype(bf16),
            "mw1t": mw1t,
            "mw2t": mw2t,
            "bc": bc,
        })
    return in_maps


def run(inputs, trace=False, trace_cores=None, tmpdir=None):
    nc = _get_nc()
    in_maps = _prep_in_maps(**inputs)
    res = bass_utils.run_bass_kernel_spmd(
        nc, in_maps, core_ids=list(range(NCORES)), trace=trace,
        trace_cores=trace_cores, tmpdir=tmpdir)
    acc = res.results[0]["ot"].astype(np.float64)
    for c in range(1, NCORES):
        acc = acc + res.results[c]["ot"].astype(np.float64)
    return acc.astype(np.float32), res


def kernel(**inputs):
    out, _ = run(inputs, trace=False)
    return out
